# revision 30
# baseline (speedup 1.0000x reference)
"""EEND-SS loss device kernel (raw Bass, SPMD over 8 cores).

Device computes, per core (B_LOC=4 samples):
  - "gram":  Gram matrix of [sep rows(12) | src rows(12) | ones] over T,
             chunk-blocked so host extracts all pairwise dots / sums / sq-sums.
  - "dgram": Gram of [logp | log1mp] x [tgt | 1-tgt] over T_sub=1000 for the
             diarization BCE (labels nearest-neighbor subsampled on the fly
             via strided DMA).
Host does the tiny O(B) permutation-invariant (PIT) math + existence BCE.

Pipeline per phase (T split into NPH column-phases of W chunks each):
  DMA (row-major, full-rate)  ->  staging[ph%3]   [128, 24*W]
  repack (DVE/ACT/GPSIMD 8 rows each) -> blocked[ph%4]  [128, 26*W]
  PE matmuls on blocked (lhsT must be a single contiguous free dim)

Blocked layout: free = (g, r, c): index = 130*g + 5*r + c, r in 0..25
 (rows 0..23 = data row r = s*6 + t*3 + i, row 24 = ones, row 25 = pad),
 c in 0..4, chunk = 5*g + c.  Partition p holds T positions
 [TCOLS*p, TCOLS*(p+1)), chunk n is column n of that view.

  scheme "fp32": per block g one matmul, lhsT = rhs = blocked[:, 130g:130g+125]
      psum_gram[125,125] += lhsT.T @ rhs ; m = 5*r + c
      host: Gamma[ra, rb] = sum_c psum[5*ra+c, 5*rb+c]

  scheme "f32r": per supergroup g2 (blocks 2g2, 2g2+1), two float32r matmuls
      with N=260 >= 256 for 1 cycle/row:
      psum_a += blocked[:, 260g2      : 260g2+125].T @ rhs(26 rows x 10 chunks)
      psum_b += blocked[:, 260g2+130 : 260g2+255].T @ rhs
      rhs free dims ordered (r, h, c): n = r*10 + 5h + c
"""

import numpy as np
from contextlib import ExitStack

import concourse.bass as bass
from concourse import mybir

F32 = mybir.dt.float32
F32R = mybir.dt.float32r
BF16 = mybir.dt.bfloat16
AFT = mybir.ActivationFunctionType

C = 3
B_LOC = 4
P = 128
PD = 125           # diar partition count
TSUB = 1000
KSUB = TSUB // PD  # 8 t_sub positions per partition
NROW = 26          # 24 data rows + ones(24) + pad(25)
NDATA = 24
BLK = 5 * NROW     # 130: free elems per chunk-group block
N_STAGE = 4
N_BLOCK = 5


def build_nc(T=128000, NPH=8, scheme="fp32", TF_STRIDE=8, dual_ring=True):
    """Build the per-core Bass program. Returns (nc, meta)."""
    TCOLS = T // P
    assert TCOLS * P == T
    W = TCOLS // NPH
    assert W * NPH == TCOLS
    if scheme in ("fp32", "bf16"):
        assert W % 5 == 0
    else:
        assert W % 10 == 0
    blk_dt = BF16 if scheme == "bf16" else F32
    G = W // 5          # blocks per phase
    TF = TSUB * TF_STRIDE
    n_stage = min(N_STAGE, NPH)
    n_block = min(N_BLOCK, NPH)

    nc = bass.Bass(trn_type="TRN2", target_bir_lowering=False, debug=False)

    sep = nc.dram_tensor("sep", [B_LOC, C, T], F32, kind="ExternalInput").ap()
    src = nc.dram_tensor("src", [B_LOC, C, T], F32, kind="ExternalInput").ap()
    diar = nc.dram_tensor("diar", [B_LOC, TSUB, C], F32, kind="ExternalInput").ap()
    lab = nc.dram_tensor("lab", [B_LOC, TF, C], F32, kind="ExternalInput").ap()

    if scheme in ("fp32", "bf16"):
        gram_out = nc.dram_tensor("gram", [PD, PD + 27], F32, kind="ExternalOutput").ap()
    else:
        gram_a_out = nc.dram_tensor("gram_a", [PD, 260], F32, kind="ExternalOutput").ap()
        gram_b_out = nc.dram_tensor("gram_b", [PD, 260], F32, kind="ExternalOutput").ap()
    if scheme == "f32r":
        dgram_out = nc.dram_tensor("dgram", [NDATA, NDATA], F32, kind="ExternalOutput").ap()

    # SBUF
    stg = [nc.alloc_sbuf_tensor(f"stg{i}", [P, NDATA * W], F32).ap()
           for i in range(n_stage)]
    blk = [nc.alloc_sbuf_tensor(f"blk{i}", [P, NROW * W], blk_dt).ap()
           for i in range(n_block)]
    p0 = nc.alloc_sbuf_tensor("p0", [PD, B_LOC * KSUB * C], F32).ap()   # (s,k,j)
    lf = nc.alloc_sbuf_tensor("lf", [PD, B_LOC * (TF // PD) * C], F32).ap()  # full labels
    # ll/rr layout: free = (k, q, s, j): k-slice contiguous 24 for matmul lhsT
    ll = nc.alloc_sbuf_tensor("ll", [PD, KSUB * 2 * B_LOC * C], F32).ap()
    rr = nc.alloc_sbuf_tensor("rr", [PD, KSUB * 2 * B_LOC * C], F32).ap()
    if scheme in ("fp32", "bf16"):
        out_sb = nc.alloc_sbuf_tensor("out_sb", [PD, PD + 27], F32).ap()
    else:
        out_sb_a = nc.alloc_sbuf_tensor("out_sb_a", [PD, 260], F32).ap()
        out_sb_b = nc.alloc_sbuf_tensor("out_sb_b", [PD, 260], F32).ap()
    d_sb = nc.alloc_sbuf_tensor("d_sb", [NDATA, NDATA], F32).ap()

    # PSUM
    if scheme in ("fp32", "bf16"):
        ps_g = nc.alloc_psum_tensor("ps_g", [PD, PD], F32).ap()
    else:
        ps_a = nc.alloc_psum_tensor("ps_a", [PD, 260], F32).ap()
        ps_b = nc.alloc_psum_tensor("ps_b", [PD, 260], F32).ap()
    ps_d = nc.alloc_psum_tensor("ps_d", [NDATA, NDATA], F32).ap()

    # helper views
    def stg3(i):
        return stg[i].rearrange("p (r n) -> p r n", r=NDATA)

    def stg4(i):  # [p, r, g, c]
        return stg[i].rearrange("p (r g c) -> p r g c", r=NDATA, g=G)

    def blk4(i):  # [p, r, g, c] view of blocked (g, r, c) layout
        return blk[i].rearrange("p (g r c) -> p r g c", g=G, r=NROW)

    # repack row split across the three copy engines
    ROWS = {"dve": (0, 17), "act": (17, 24)}

    with ExitStack() as ctx:
        st_sems = [ctx.enter_context(nc.semaphore(f"st_sem{i}")) for i in range(NPH)]
        pdma_sem = ctx.enter_context(nc.semaphore("pdma_sem"))
        ldma_sem = ctx.enter_context(nc.semaphore("ldma_sem"))
        odma_sem = ctx.enter_context(nc.semaphore("odma_sem"))
        rpk_dve = ctx.enter_context(nc.semaphore("rpk_dve"))
        rpk_act = ctx.enter_context(nc.semaphore("rpk_act"))
        rpk_gp = ctx.enter_context(nc.semaphore("rpk_gp"))
        act_sem = ctx.enter_context(nc.semaphore("act_sem"))
        dve_sem = ctx.enter_context(nc.semaphore("dve_sem"))
        pe_sem = ctx.enter_context(nc.semaphore("pe_sem"))
        block = ctx.enter_context(nc.Block())

        def issue_phase_dmas(eng, ph, samples):
            s3 = stg3(ph % n_stage)
            for s in samples:
                for t, big in enumerate((sep, src)):
                    r0 = s * 6 + t * 3
                    src_ap = big[s].rearrange("i (p n) -> p i n", p=P)[
                        :, :, ph * W:(ph + 1) * W]
                    eng.dma_start(out=s3[:, r0:r0 + 3, :], in_=src_ap
                                  ).then_inc(st_sems[ph], 16)

        @block.sync
        def _(sync: bass.BassEngine):
            # big data split across both HWDGE rings (desc-gen is the issue-rate
            # bottleneck: ~0.8us per DMA, serialized per ring)
            for ph in range(NPH):
                if ph >= n_stage:
                    # WAR: staging slot reused; wait repack of ph - n_stage done
                    v = ph - n_stage + 1
                    sync.wait_ge(rpk_dve, v)
                    sync.wait_ge(rpk_act, v)
                issue_phase_dmas(sync, ph, (0, 1) if dual_ring else (0, 1, 2, 3))

            # outputs at the very end
            sync.wait_ge(dve_sem, 4)
            if scheme in ("fp32", "bf16"):
                sync.dma_start(out=gram_out, in_=out_sb).then_inc(odma_sem, 16)
                n_out = 1
            else:
                sync.dma_start(out=gram_a_out, in_=out_sb_a).then_inc(odma_sem, 16)
                sync.dma_start(out=gram_b_out, in_=out_sb_b).then_inc(odma_sem, 16)
                sync.dma_start(out=dgram_out, in_=d_sb).then_inc(odma_sem, 16)
                n_out = 3
            sync.wait_ge(odma_sem, 16 * n_out)

        def out_rpk(ph):
            v = blk4(ph % n_block)
            return v.bitcast(F32R) if scheme == "f32r" else v

        def repack(eng, sem, rows, copy_fn, mid_hook=None):
            r0, r1 = rows
            for ph in range(NPH):
                eng.wait_ge(st_sems[ph], 16 * 8)
                if ph >= n_block:
                    eng.wait_ge(pe_sem, ph - n_block + 1)
                copy_fn(
                    out_rpk(ph)[:, r0:r1, :, :],
                    stg4(ph % n_stage)[:, r0:r1, :, :],
                ).then_inc(sem, 1)
                if mid_hook is not None and ph == mid_hook[0]:
                    mid_hook[1]()

        @block.gpsimd
        def _(gpsimd: bass.BassEngine):
            if scheme in ("fp32", "bf16"):
                gpsimd.memset(out_sb, 0.0).then_inc(rpk_gp, 1)
            # ones(row 24) + pad(row 25) once per blocked slot; no repack here
            # (gpsimd copies measured ~6x slower than DVE)
            for i in range(n_block):
                ap1 = blk[i].rearrange("p (g x) -> p g x", g=G)[:, :, 5 * NDATA:5 * NROW]
                if scheme == "f32r":
                    ap1 = ap1.bitcast(F32R)
                gpsimd.memset(ap1, 1.0).then_inc(rpk_gp, 1)

        @block.scalar
        def _(scalar: bass.BassEngine):
            def diar_dmas():
                # diar inputs on the scalar HWDGE ring, contiguous layouts only
                # (12B-granule gathers would grind the SDMA engines for ~30us)
                scalar.dma_start(
                    out=p0.rearrange("p (s x) -> p s x", s=B_LOC),
                    in_=diar.rearrange("s (p k) j -> p s (k j)", p=PD),
                ).then_inc(pdma_sem, 16)
                scalar.dma_start(
                    out=lf.rearrange("p (s x) -> p s x", s=B_LOC),
                    in_=lab.rearrange("s (p e) j -> p s (e j)", p=PD),
                ).then_inc(ldma_sem, 16)

            if dual_ring:
                issue_phase_dmas(scalar, 0, (2, 3))
                if NPH > 1:
                    issue_phase_dmas(scalar, 1, (2, 3))
            diar_dmas()
            # per phase: repack this phase, then issue phase ph+2's DMAs
            # (ph+1 already issued) so repack(0) is never starved
            r0, r1 = ROWS["act"]

            def diar_acts():
                rrk = rr.rearrange("p (k q s j) -> p k q s j", k=KSUB, q=2, s=B_LOC)
                llk = ll.rearrange("p (k q s j) -> p k q s j", k=KSUB, q=2, s=B_LOC)
                p0k = p0.rearrange("p (s k j) -> p k s j", s=B_LOC, k=KSUB)
                scalar.wait_ge(pdma_sem, 16)
                scalar.activation(llk[:, :, 0, :, :], p0k, AFT.Ln).then_inc(act_sem, 1)
                scalar.activation(llk[:, :, 1, :, :], p0k, AFT.Ln,
                                  scale=-1.0, bias=1.0).then_inc(act_sem, 1)
                scalar.wait_ge(dve_sem, 1)
                scalar.activation(rrk[:, :, 1, :, :], rrk[:, :, 0, :, :], AFT.Copy,
                                  scale=-1.0, bias=1.0).then_inc(act_sem, 1)

            for ph in range(NPH):
                scalar.wait_ge(st_sems[ph], 16 * 8)
                if ph >= n_block:
                    scalar.wait_ge(pe_sem, ph - n_block + 1)
                scalar.activation(
                    out_rpk(ph)[:, r0:r1, :, :],
                    stg4(ph % n_stage)[:, r0:r1, :, :],
                    AFT.Copy).then_inc(rpk_act, 1)
                if ph == min(1, NPH - 1):
                    diar_acts()
                if dual_ring and ph + 2 < NPH:
                    if ph + 2 >= n_stage:
                        scalar.wait_ge(rpk_dve, ph + 2 - n_stage + 1)
                        scalar.wait_ge(rpk_act, ph + 2 - n_stage + 1)
                    issue_phase_dmas(scalar, ph + 2, (2, 3))

        @block.vector
        def _(vector: bass.BassEngine):
            def diar_dve():
                # nearest-neighbor label subsample: lf (s, 8k+f, j), f=0
                rrk = rr.rearrange("p (k q s j) -> p k q s j", k=KSUB, q=2, s=B_LOC)
                lf5 = lf.rearrange("p (s k f j) -> p k s f j", s=B_LOC, k=KSUB,
                                   f=(TF // PD) // KSUB)[:, :, :, 0, :]
                vector.wait_ge(ldma_sem, 16)
                vector.tensor_copy(rrk[:, :, 0, :, :], lf5).then_inc(dve_sem, 1)
                vector.wait_ge(act_sem, 2)
                vector.tensor_scalar_max(ll[:, :], ll[:, :], -100.0).then_inc(dve_sem, 1)

            repack(vector, rpk_dve, ROWS["dve"], vector.tensor_copy,
                   mid_hook=(min(0, NPH - 1), diar_dve))
            if scheme in ("fp32", "bf16"):
                vector.wait_ge(pe_sem, min(2, NPH - 1) + 2)
                vector.wait_ge(rpk_gp, 1)
                vector.tensor_copy(out_sb[0:NDATA, PD:PD + NDATA], ps_d
                                   ).then_inc(dve_sem, 1)
                vector.wait_ge(pe_sem, NPH + 1)
                vector.tensor_copy(out_sb[:, 0:PD], ps_g).then_inc(dve_sem, 1)
            else:
                vector.tensor_copy(out_sb_a, ps_a)
                vector.tensor_copy(out_sb_b, ps_b).then_inc(dve_sem, 1)
                vector.wait_ge(pe_sem, NPH + 1)
                vector.tensor_copy(d_sb, ps_d).then_inc(dve_sem, 1)

        @block.tensor
        def _(tensor: bass.BassEngine):
            nmm = 0
            if scheme in ("fp32", "bf16"):
                total_mm = NPH * G
            else:
                total_mm = NPH * (G // 2) * 2
            for ph in range(NPH):
                v = ph + 1
                tensor.wait_ge(rpk_dve, v)
                tensor.wait_ge(rpk_act, v)
                tensor.wait_ge(rpk_gp, n_block + (1 if scheme in ('fp32', 'bf16') else 0))
                b = blk[ph % n_block]
                if scheme in ("fp32", "bf16"):
                    for g in range(G):
                        ap = b[:, BLK * g: BLK * g + 125]
                        mm = tensor.matmul(ps_g, ap, ap,
                                           start=(nmm == 0), stop=(nmm == total_mm - 1))
                        nmm += 1
                else:
                    b5 = b.rearrange("p (G h r c) -> p G r h c", h=2, r=NROW, c=5)
                    for g2 in range(G // 2):
                        rhs = b5[:, g2].bitcast(F32R)        # [p, 26, 2, 5]
                        la = b[:, 2 * BLK * g2: 2 * BLK * g2 + 125].bitcast(F32R)
                        lb = b[:, 2 * BLK * g2 + BLK: 2 * BLK * g2 + BLK + 125].bitcast(F32R)
                        first = nmm == 0
                        last = nmm == total_mm - 2
                        mm = tensor.matmul(ps_a, la, rhs, start=first, stop=last)
                        nmm += 1
                        mm = tensor.matmul(ps_b, lb, rhs, start=first, stop=last)
                        nmm += 1
                mm.then_inc(pe_sem, 1)
                if ph == min(2, NPH - 1):
                    # diar matmuls mid-stream; lhsT k-slices contiguous 24 cols
                    tensor.wait_ge(pdma_sem, 16)
                    tensor.wait_ge(ldma_sem, 16)
                    tensor.wait_ge(act_sem, 3)
                    tensor.wait_ge(dve_sem, 2)
                    nd = 2 * B_LOC * C  # 24
                    for k in range(KSUB):
                        dmm = tensor.matmul(ps_d, ll[:, k * nd:(k + 1) * nd],
                                            rr[:, k * nd:(k + 1) * nd],
                                            start=(k == 0), stop=(k == KSUB - 1))
                    dmm.then_inc(pe_sem, 1)

    meta = dict(T=T, NPH=NPH, W=W, scheme=scheme, dual_ring=dual_ring)
    return nc, meta


def build_nc_v2(T=128000, PHW=(260, 240, 190, 180, 130)):
    """v2 schedule: all staging resident in SBUF (no WAR reuse), DMA
    descriptor-gen split across sync (samples 0,1) + gpsimd (samples 2,3)
    rings and issued fully upfront, diar chain front-loaded, repack split
    DVE rows 0-12 / ACT rows 13-23, non-uniform column phases (big early,
    small last) to shrink the post-stream tail. bf16 blocked Gram scheme,
    output layout identical to build_nc(scheme='fp32'/'bf16')."""
    TCOLS = T // P
    assert TCOLS * P == T
    assert sum(PHW) == TCOLS
    for w in PHW:
        assert w % 5 == 0 and w >= 128
    NPH = len(PHW)
    COFF = [sum(PHW[:i]) for i in range(NPH)]      # column offsets
    G0 = [c // 5 for c in COFF]                    # block offsets
    GS = [w // 5 for w in PHW]                     # blocks per phase
    G_TOT = TCOLS // 5
    TF = TSUB * 8

    # repack row split (blk rows 0..23 data, 24 ones, 25 pad)
    DVE_ROWS = (0, 13)
    ACT_ROWS = (13, 24)

    nc = bass.Bass(trn_type="TRN2", target_bir_lowering=False, debug=False)

    sep = nc.dram_tensor("sep", [B_LOC, C, T], F32, kind="ExternalInput").ap()
    src = nc.dram_tensor("src", [B_LOC, C, T], F32, kind="ExternalInput").ap()
    diar = nc.dram_tensor("diar", [B_LOC, TSUB, C], F32, kind="ExternalInput").ap()
    lab = nc.dram_tensor("lab", [B_LOC, TF, C], F32, kind="ExternalInput").ap()
    gram_out = nc.dram_tensor("gram", [PD, PD + 27], F32, kind="ExternalOutput").ap()

    # SBUF: everything resident, no slot reuse
    stg = nc.alloc_sbuf_tensor("stg", [P, NDATA * TCOLS], F32).ap()
    blk = nc.alloc_sbuf_tensor("blk", [P, NROW * TCOLS], BF16).ap()
    p0 = nc.alloc_sbuf_tensor("p0", [PD, B_LOC * KSUB * C], F32).ap()
    lf = nc.alloc_sbuf_tensor("lf", [PD, B_LOC * (TF // PD) * C], F32).ap()
    ll = nc.alloc_sbuf_tensor("ll", [PD, KSUB * 2 * B_LOC * C], F32).ap()
    rr = nc.alloc_sbuf_tensor("rr", [PD, KSUB * 2 * B_LOC * C], F32).ap()
    out_sb = nc.alloc_sbuf_tensor("out_sb", [PD, PD + 27], F32).ap()

    ps_g = nc.alloc_psum_tensor("ps_g", [PD, PD], F32).ap()
    ps_d = nc.alloc_psum_tensor("ps_d", [NDATA, NDATA], F32).ap()

    stg3 = stg.rearrange("p (r n) -> p r n", r=NDATA)
    stg4 = stg.rearrange("p (r g c) -> p r g c", r=NDATA, g=G_TOT)
    blk4 = blk.rearrange("p (g r c) -> p r g c", g=G_TOT, r=NROW)

    with ExitStack() as ctx:
        st = [ctx.enter_context(nc.semaphore(f"st{i}")) for i in range(NPH)]
        pdma_sem = ctx.enter_context(nc.semaphore("pdma_sem"))
        ldma_sem = ctx.enter_context(nc.semaphore("ldma_sem"))
        odma_sem = ctx.enter_context(nc.semaphore("odma_sem"))
        gp_init = ctx.enter_context(nc.semaphore("gp_init"))
        act_sem = ctx.enter_context(nc.semaphore("act_sem"))
        dve_sem = ctx.enter_context(nc.semaphore("dve_sem"))
        pe_sem = ctx.enter_context(nc.semaphore("pe_sem"))
        rpk_dve = ctx.enter_context(nc.semaphore("rpk_dve"))
        rpk_act = ctx.enter_context(nc.semaphore("rpk_act"))
        block = ctx.enter_context(nc.Block())

        def issue_phase_dmas(eng, ph, samples):
            c0, w = COFF[ph], PHW[ph]
            for s in samples:
                for t, big in enumerate((sep, src)):
                    r0 = s * 6 + t * 3
                    src_ap = big[s].rearrange("i (p n) -> p i n", p=P)[
                        :, :, c0:c0 + w]
                    eng.dma_start(out=stg3[:, r0:r0 + 3, c0:c0 + w], in_=src_ap
                                  ).then_inc(st[ph], 16)

        @block.sync
        def _(sync: bass.BassEngine):
            # entire ring issued upfront, no waits: staging is fully resident
            for ph in range(NPH):
                issue_phase_dmas(sync, ph, (0, 1))
            sync.wait_ge(dve_sem, 4)
            sync.dma_start(out=gram_out, in_=out_sb).then_inc(odma_sem, 16)
            sync.wait_ge(odma_sem, 16)

        @block.gpsimd
        def _(gpsimd: bass.BassEngine):
            gpsimd.memset(out_sb, 0.0).then_inc(gp_init, 1)
            ones_ap = blk.rearrange("p (g x) -> p g x", g=G_TOT)[:, :, 5 * NDATA:5 * NROW]
            gpsimd.memset(ones_ap, 1.0).then_inc(gp_init, 1)
            for ph in range(NPH):
                issue_phase_dmas(gpsimd, ph, (2, 3))

        @block.scalar
        def _(scalar: bass.BassEngine):
            scalar.dma_start(
                out=p0.rearrange("p (s x) -> p s x", s=B_LOC),
                in_=diar.rearrange("s (p k) j -> p s (k j)", p=PD),
            ).then_inc(pdma_sem, 16)
            scalar.dma_start(
                out=lf.rearrange("p (s x) -> p s x", s=B_LOC),
                in_=lab.rearrange("s (p e) j -> p s (e j)", p=PD),
            ).then_inc(ldma_sem, 16)

            rrk = rr.rearrange("p (k q s j) -> p k q s j", k=KSUB, q=2, s=B_LOC)
            llk = ll.rearrange("p (k q s j) -> p k q s j", k=KSUB, q=2, s=B_LOC)
            p0k = p0.rearrange("p (s k j) -> p k s j", s=B_LOC, k=KSUB)
            scalar.wait_ge(pdma_sem, 16)
            scalar.activation(llk[:, :, 0, :, :], p0k, AFT.Ln).then_inc(act_sem, 1)
            scalar.activation(llk[:, :, 1, :, :], p0k, AFT.Ln,
                              scale=-1.0, bias=1.0).then_inc(act_sem, 1)
            scalar.wait_ge(dve_sem, 1)
            scalar.activation(rrk[:, :, 1, :, :], rrk[:, :, 0, :, :], AFT.Copy,
                              scale=-1.0, bias=1.0).then_inc(act_sem, 1)

            r0, r1 = ACT_ROWS
            for ph in range(NPH):
                scalar.wait_ge(st[ph], 16 * 8)
                scalar.activation(
                    blk4[:, r0:r1, G0[ph]:G0[ph] + GS[ph], :],
                    stg4[:, r0:r1, G0[ph]:G0[ph] + GS[ph], :],
                    AFT.Copy).then_inc(rpk_act, 1)

        @block.vector
        def _(vector: bass.BassEngine):
            rrk = rr.rearrange("p (k q s j) -> p k q s j", k=KSUB, q=2, s=B_LOC)
            lf5 = lf.rearrange("p (s k f j) -> p k s f j", s=B_LOC, k=KSUB,
                               f=(TF // PD) // KSUB)[:, :, :, 0, :]
            vector.wait_ge(ldma_sem, 16)
            vector.tensor_copy(rrk[:, :, 0, :, :], lf5).then_inc(dve_sem, 1)
            vector.wait_ge(act_sem, 2)
            vector.tensor_scalar_max(ll[:, :], ll[:, :], -100.0).then_inc(dve_sem, 1)
            vector.wait_ge(gp_init, 1)
            vector.wait_ge(pe_sem, 1)
            vector.tensor_copy(out_sb[0:NDATA, PD:PD + NDATA], ps_d
                               ).then_inc(dve_sem, 1)
            r0, r1 = DVE_ROWS
            for ph in range(NPH):
                vector.wait_ge(st[ph], 16 * 8)
                vector.tensor_copy(
                    blk4[:, r0:r1, G0[ph]:G0[ph] + GS[ph], :],
                    stg4[:, r0:r1, G0[ph]:G0[ph] + GS[ph], :],
                ).then_inc(rpk_dve, 1)
            vector.wait_ge(pe_sem, NPH + 1)
            vector.tensor_copy(out_sb[:, 0:PD], ps_g).then_inc(dve_sem, 1)

        @block.tensor
        def _(tensor: bass.BassEngine):
            # diar matmuls first: everything ready by ~15us
            tensor.wait_ge(act_sem, 3)
            tensor.wait_ge(dve_sem, 2)
            nd = 2 * B_LOC * C  # 24
            for k in range(KSUB):
                dmm = tensor.matmul(ps_d, ll[:, k * nd:(k + 1) * nd],
                                    rr[:, k * nd:(k + 1) * nd],
                                    start=(k == 0), stop=(k == KSUB - 1))
            dmm.then_inc(pe_sem, 1)

            tensor.wait_ge(gp_init, 2)
            nmm = 0
            for ph in range(NPH):
                tensor.wait_ge(rpk_dve, ph + 1)
                tensor.wait_ge(rpk_act, ph + 1)
                for g in range(G0[ph], G0[ph] + GS[ph]):
                    ap = blk[:, BLK * g: BLK * g + 125]
                    mm = tensor.matmul(ps_g, ap, ap,
                                       start=(nmm == 0), stop=(nmm == G_TOT - 1))
                    nmm += 1
                mm.then_inc(pe_sem, 1)

    return nc, dict(T=T, PHW=PHW)


def build_nc_v7(T=128000, PHW=(260, 250, 230, 130, 130)):
    """v7: the gpsimd SWDGE queue (Q0) gets drained with priority by the
    DMA engines — exploit it: Q0 carries the diar inputs + ALL of ph0
    (trickled per (tensor,sample)), landing early while the two HWDGE
    rings deliver ph1-ph4 with the leftover bandwidth; the total stays
    work-conserving at ~350 GB/s and the rings never host the small or
    odd-pattern DMAs that starve descriptor supply. ps_d copy sits late on
    DVE (off the cast path). PE: diar, ph0..ph4 back-to-back, HAM-warm."""
    TCOLS = T // P
    assert TCOLS * P == T
    assert sum(PHW) == TCOLS
    for w in PHW:
        assert w % 5 == 0 and w >= 128
    NPH = 5
    COFF = [sum(PHW[:i]) for i in range(NPH)]
    G0 = [c // 5 for c in COFF]
    GS = [w // 5 for w in PHW]
    G_TOT = TCOLS // 5
    TF = TSUB * 8
    H2 = GS[2] // 2

    nc = bass.Bass(trn_type="TRN2", target_bir_lowering=False, debug=False)

    sep = nc.dram_tensor("sep", [B_LOC, C, T], F32, kind="ExternalInput").ap()
    src = nc.dram_tensor("src", [B_LOC, C, T], F32, kind="ExternalInput").ap()
    diar = nc.dram_tensor("diar", [B_LOC, TSUB, C], F32, kind="ExternalInput").ap()
    lab = nc.dram_tensor("lab", [B_LOC, TF, C], F32, kind="ExternalInput").ap()
    gram_out = nc.dram_tensor("gram", [PD, PD + 27], F32, kind="ExternalOutput").ap()

    stg = nc.alloc_sbuf_tensor("stg", [P, NDATA * TCOLS], F32).ap()
    blk = nc.alloc_sbuf_tensor("blk", [P, NROW * TCOLS], BF16).ap()
    p0 = nc.alloc_sbuf_tensor("p0", [PD, B_LOC * KSUB * C], F32).ap()
    lf = nc.alloc_sbuf_tensor("lf", [PD, B_LOC * (TF // PD) * C], F32).ap()
    ll = nc.alloc_sbuf_tensor("ll", [PD, KSUB * 2 * B_LOC * C], F32).ap()
    rr = nc.alloc_sbuf_tensor("rr", [PD, KSUB * 2 * B_LOC * C], F32).ap()
    out_sb = nc.alloc_sbuf_tensor("out_sb", [PD, PD + 27], F32).ap()

    ps_g = nc.alloc_psum_tensor("ps_g", [P, PD], F32).ap()
    ps_d = nc.alloc_psum_tensor("ps_d", [NDATA, NDATA], F32).ap()

    stg3 = stg.rearrange("p (r n) -> p r n", r=NDATA)
    stg4 = stg.rearrange("p (r g c) -> p r g c", r=NDATA, g=G_TOT)
    blk4 = blk.rearrange("p (g r c) -> p r g c", g=G_TOT, r=NROW)

    with ExitStack() as ctx:
        stA = [ctx.enter_context(nc.semaphore(f"stA{i}")) for i in range(3)]
        stB = [ctx.enter_context(nc.semaphore(f"stB{i}")) for i in range(3)]
        stQ = [ctx.enter_context(nc.semaphore(f"stQ{i}")) for i in range(8)]
        stA3 = [ctx.enter_context(nc.semaphore(f"stA3_{i}")) for i in range(B_LOC)]
        stB3 = [ctx.enter_context(nc.semaphore(f"stB3_{i}")) for i in range(B_LOC)]
        stA4 = [ctx.enter_context(nc.semaphore(f"stA4_{i}")) for i in range(B_LOC)]
        stB4 = [ctx.enter_context(nc.semaphore(f"stB4_{i}")) for i in range(B_LOC)]
        pdma_sem = ctx.enter_context(nc.semaphore("pdma_sem"))
        ldma_sem = ctx.enter_context(nc.semaphore("ldma_sem"))
        odma_sem = ctx.enter_context(nc.semaphore("odma_sem"))
        gp_init = ctx.enter_context(nc.semaphore("gp_init"))
        act_sem = ctx.enter_context(nc.semaphore("act_sem"))
        dve_sem = ctx.enter_context(nc.semaphore("dve_sem"))
        pe_sem = ctx.enter_context(nc.semaphore("pe_sem"))
        rpk_dve = ctx.enter_context(nc.semaphore("rpk_dve"))
        rpk_act = ctx.enter_context(nc.semaphore("rpk_act"))
        block = ctx.enter_context(nc.Block())

        def phase6(eng, big, row0, sem, ph):
            c0, w = COFF[ph], PHW[ph]
            for s0 in (0, 2):
                r = row0 + 3 * s0
                eng.dma_start(
                    out=stg3[:, r:r + 6, c0:c0 + w],
                    in_=big[s0:s0 + 2].rearrange("s i (p n) -> p (s i) n", p=P)[
                        :, :, c0:c0 + w],
                ).then_inc(sem[ph], 16)

        def trickle_descs(eng, big, row0, subsems, ph):
            c0, w = COFF[ph], PHW[ph]
            for s in range(B_LOC):
                r = row0 + 3 * s
                eng.dma_start(
                    out=stg3[:, r:r + 3, c0:c0 + w],
                    in_=big[s].rearrange("i (p n) -> p i n", p=P)[:, :, c0:c0 + w],
                ).then_inc(subsems[s], 16)

        @block.sync
        def _(sync: bass.BassEngine):
            phase6(sync, sep, 0, stA, 1)
            phase6(sync, sep, 0, stA, 2)
            trickle_descs(sync, sep, 0, stA3, 3)
            trickle_descs(sync, sep, 0, stA4, 4)
            sync.wait_ge(dve_sem, 4)
            sync.dma_start(out=gram_out, in_=out_sb).then_inc(odma_sem, 16)
            sync.wait_ge(odma_sem, 16)

        @block.gpsimd
        def _(gpsimd: bass.BassEngine):
            gpsimd.dma_start(
                out=p0.rearrange("p (s x) -> p s x", s=B_LOC),
                in_=diar.rearrange("s (p k) j -> p s (k j)", p=PD),
            ).then_inc(pdma_sem, 16)
            gpsimd.dma_start(
                out=lf.rearrange("p (s x) -> p s x", s=B_LOC),
                in_=lab.rearrange("s (p e) j -> p s (e j)", p=PD),
            ).then_inc(ldma_sem, 16)
            gpsimd.memset(out_sb, 0.0).then_inc(gp_init, 1)
            ones_ap = blk.rearrange("p (g x) -> p g x", g=G_TOT)[:, :, 5 * NDATA:5 * NROW]
            gpsimd.memset(ones_ap, 1.0).then_inc(gp_init, 1)
            # ph0 per (sample, tensor): sem index q = s*2 + t
            c0, w = COFF[0], PHW[0]
            for s in range(B_LOC):
                for t, big in enumerate((sep, src)):
                    r = 12 * t + 3 * s
                    gpsimd.dma_start(
                        out=stg3[:, r:r + 3, c0:c0 + w],
                        in_=big[s].rearrange("i (p n) -> p i n", p=P)[:, :, c0:c0 + w],
                    ).then_inc(stQ[s * 2 + t], 16)

        @block.scalar
        def _(scalar: bass.BassEngine):
            phase6(scalar, src, 12, stB, 1)

            rrk = rr.rearrange("p (k q s j) -> p k q s j", k=KSUB, q=2, s=B_LOC)
            llk = ll.rearrange("p (k q s j) -> p k q s j", k=KSUB, q=2, s=B_LOC)
            p0k = p0.rearrange("p (s k j) -> p k s j", s=B_LOC, k=KSUB)
            scalar.wait_ge(pdma_sem, 16)
            scalar.activation(llk[:, :, 0, :, :], p0k, AFT.Ln).then_inc(act_sem, 1)
            scalar.activation(llk[:, :, 1, :, :], p0k, AFT.Ln,
                              scale=-1.0, bias=1.0).then_inc(act_sem, 1)
            scalar.wait_ge(dve_sem, 1)
            scalar.activation(rrk[:, :, 1, :, :], rrk[:, :, 0, :, :], AFT.Copy,
                              scale=-1.0, bias=1.0).then_inc(act_sem, 1)

            phase6(scalar, src, 12, stB, 2)
            trickle_descs(scalar, src, 12, stB3, 3)
            trickle_descs(scalar, src, 12, stB4, 4)

            def achunk(r0, subsems, i0, i1, ph):
                gl = slice(G0[ph], G0[ph] + GS[ph])
                scalar.wait_ge(subsems[i0], 16)
                scalar.wait_ge(subsems[i1], 16)
                scalar.activation(
                    blk4[:, r0:r0 + 6, gl, :],
                    stg4[:, r0:r0 + 6, gl, :],
                    AFT.Copy).then_inc(rpk_act, 1)

            achunk(12, stB3, 0, 1, 3)
            achunk(12, stB4, 0, 1, 4)

        @block.vector
        def _(vector: bass.BassEngine):
            def cast(r0, r1, g0, g1):
                vector.tensor_copy(
                    blk4[:, r0:r1, g0:g1, :],
                    stg4[:, r0:r1, g0:g1, :],
                ).then_inc(rpk_dve, 1)

            rrk = rr.rearrange("p (k q s j) -> p k q s j", k=KSUB, q=2, s=B_LOC)
            lf5 = lf.rearrange("p (s k f j) -> p k s f j", s=B_LOC, k=KSUB,
                               f=(TF // PD) // KSUB)[:, :, :, 0, :]
            vector.wait_ge(ldma_sem, 16)
            vector.tensor_copy(rrk[:, :, 0, :, :], lf5).then_inc(dve_sem, 1)
            vector.wait_ge(act_sem, 2)
            vector.tensor_scalar_max(ll[:, :], ll[:, :], -100.0).then_inc(dve_sem, 1)
            # ph0 chunk casts as Q0 sub-DMAs land (sem q = s*2 + t)
            gl0 = (G0[0], G0[0] + GS[0])
            for s in range(B_LOC):
                for t in range(2):
                    r = 12 * t + 3 * s
                    vector.wait_ge(stQ[s * 2 + t], 16)
                    cast(r, r + 3, gl0[0], gl0[1])
            vector.wait_ge(stA[1], 32)
            vector.wait_ge(stB[1], 32)
            cast(0, 24, G0[1], G0[1] + GS[1])
            vector.wait_ge(stA[2], 32)
            vector.wait_ge(stB[2], 32)
            cast(0, 24, G0[2], G0[2] + H2)
            cast(0, 24, G0[2] + H2, G0[2] + GS[2])

            vector.wait_ge(gp_init, 1)
            vector.wait_ge(pe_sem, 1)
            vector.tensor_copy(out_sb[0:NDATA, PD:PD + NDATA], ps_d
                               ).then_inc(dve_sem, 1)

            def vchunk(r0, subsems, i0, i1, ph):
                gl = (G0[ph], G0[ph] + GS[ph])
                vector.wait_ge(subsems[i0], 16)
                vector.wait_ge(subsems[i1], 16)
                cast(r0, r0 + 6, gl[0], gl[1])

            vchunk(0, stA3, 0, 1, 3)
            vchunk(6, stA3, 2, 3, 3)
            vchunk(18, stB3, 2, 3, 3)
            vchunk(0, stA4, 0, 1, 4)
            vchunk(6, stA4, 2, 3, 4)
            vchunk(18, stB4, 2, 3, 4)
            vector.wait_ge(pe_sem, 6)
            vector.tensor_copy(out_sb[:, 0:PD], ps_g[0:PD, :]).then_inc(dve_sem, 1)

        @block.tensor
        def _(tensor: bass.BassEngine):
            nmm = [0]

            def mms(g0, g1):
                mm = None
                for g in range(g0, g1):
                    lhsT = blk[:, BLK * g: BLK * g + 128]
                    rhs = blk[:, BLK * g: BLK * g + 125]
                    mm = tensor.matmul(ps_g, lhsT, rhs,
                                       start=(nmm[0] == 0), stop=(nmm[0] == G_TOT - 1))
                    nmm[0] += 1
                return mm

            tensor.wait_ge(act_sem, 3)
            tensor.wait_ge(dve_sem, 2)
            nd = 2 * B_LOC * C
            for k in range(KSUB):
                dmm = tensor.matmul(ps_d, ll[:, k * nd:(k + 1) * nd],
                                    rr[:, k * nd:(k + 1) * nd],
                                    start=(k == 0), stop=(k == KSUB - 1))
            dmm.then_inc(pe_sem, 1)

            tensor.wait_ge(gp_init, 2)
            tensor.wait_ge(rpk_dve, 8)
            mms(G0[0], G0[0] + GS[0]).then_inc(pe_sem, 1)
            tensor.wait_ge(rpk_dve, 9)
            mms(G0[1], G0[1] + GS[1]).then_inc(pe_sem, 1)
            tensor.wait_ge(rpk_dve, 10)
            mms(G0[2], G0[2] + H2)
            tensor.wait_ge(rpk_dve, 11)
            mms(G0[2] + H2, G0[2] + GS[2]).then_inc(pe_sem, 1)
            tensor.wait_ge(rpk_dve, 14)
            tensor.wait_ge(rpk_act, 1)
            mms(G0[3], G0[3] + GS[3]).then_inc(pe_sem, 1)
            tensor.wait_ge(rpk_dve, 17)
            tensor.wait_ge(rpk_act, 2)
            mms(G0[4], G0[4] + GS[4]).then_inc(pe_sem, 1)

    return nc, dict(T=T, PHW=PHW, tmajor=True)


def build_nc_v6(T=128000, PHW=(260, 250, 230, 130, 130)):
    """v6: wide early phases (packets >=920B sustain ~350 GB/s; small
    packets cap lower), BOTH last phases small (130 cols) and trickled
    per-sample so repack+PE chase the drain. Casts chunked 6 rows (halves
    the per-instruction overhead on the tail). PE: diar, ph0, ph1, ph2
    halves, ph3, ph4 — ph3 lands right before ph4 to keep HAM warm."""
    TCOLS = T // P
    assert TCOLS * P == T
    assert sum(PHW) == TCOLS
    for w in PHW:
        assert w % 5 == 0 and w >= 128
    NPH = 5
    COFF = [sum(PHW[:i]) for i in range(NPH)]
    G0 = [c // 5 for c in COFF]
    GS = [w // 5 for w in PHW]
    G_TOT = TCOLS // 5
    TF = TSUB * 8
    H2 = GS[2] // 2

    nc = bass.Bass(trn_type="TRN2", target_bir_lowering=False, debug=False)

    sep = nc.dram_tensor("sep", [B_LOC, C, T], F32, kind="ExternalInput").ap()
    src = nc.dram_tensor("src", [B_LOC, C, T], F32, kind="ExternalInput").ap()
    diar = nc.dram_tensor("diar", [B_LOC, TSUB, C], F32, kind="ExternalInput").ap()
    lab = nc.dram_tensor("lab", [B_LOC, TF, C], F32, kind="ExternalInput").ap()
    gram_out = nc.dram_tensor("gram", [PD, PD + 27], F32, kind="ExternalOutput").ap()

    stg = nc.alloc_sbuf_tensor("stg", [P, NDATA * TCOLS], F32).ap()
    blk = nc.alloc_sbuf_tensor("blk", [P, NROW * TCOLS], BF16).ap()
    p0 = nc.alloc_sbuf_tensor("p0", [PD, B_LOC * KSUB * C], F32).ap()
    lf = nc.alloc_sbuf_tensor("lf", [PD, B_LOC * (TF // PD) * C], F32).ap()
    ll = nc.alloc_sbuf_tensor("ll", [PD, KSUB * 2 * B_LOC * C], F32).ap()
    rr = nc.alloc_sbuf_tensor("rr", [PD, KSUB * 2 * B_LOC * C], F32).ap()
    out_sb = nc.alloc_sbuf_tensor("out_sb", [PD, PD + 27], F32).ap()

    ps_g = nc.alloc_psum_tensor("ps_g", [P, PD], F32).ap()
    ps_d = nc.alloc_psum_tensor("ps_d", [NDATA, NDATA], F32).ap()

    stg3 = stg.rearrange("p (r n) -> p r n", r=NDATA)
    stg4 = stg.rearrange("p (r g c) -> p r g c", r=NDATA, g=G_TOT)
    blk4 = blk.rearrange("p (g r c) -> p r g c", g=G_TOT, r=NROW)

    with ExitStack() as ctx:
        stA = [ctx.enter_context(nc.semaphore(f"stA{i}")) for i in range(3)]
        stB = [ctx.enter_context(nc.semaphore(f"stB{i}")) for i in range(3)]
        stA3 = [ctx.enter_context(nc.semaphore(f"stA3_{i}")) for i in range(B_LOC)]
        stB3 = [ctx.enter_context(nc.semaphore(f"stB3_{i}")) for i in range(B_LOC)]
        stA4 = [ctx.enter_context(nc.semaphore(f"stA4_{i}")) for i in range(B_LOC)]
        stB4 = [ctx.enter_context(nc.semaphore(f"stB4_{i}")) for i in range(B_LOC)]
        pdma_sem = ctx.enter_context(nc.semaphore("pdma_sem"))
        ldma_sem = ctx.enter_context(nc.semaphore("ldma_sem"))
        odma_sem = ctx.enter_context(nc.semaphore("odma_sem"))
        gp_init = ctx.enter_context(nc.semaphore("gp_init"))
        act_sem = ctx.enter_context(nc.semaphore("act_sem"))
        dve_sem = ctx.enter_context(nc.semaphore("dve_sem"))
        pe_sem = ctx.enter_context(nc.semaphore("pe_sem"))
        rpk_dve = ctx.enter_context(nc.semaphore("rpk_dve"))
        rpk_act = ctx.enter_context(nc.semaphore("rpk_act"))
        block = ctx.enter_context(nc.Block())

        def phase6(eng, big, row0, sem, ph):
            c0, w = COFF[ph], PHW[ph]
            for s0 in (0, 2):
                r = row0 + 3 * s0
                eng.dma_start(
                    out=stg3[:, r:r + 6, c0:c0 + w],
                    in_=big[s0:s0 + 2].rearrange("s i (p n) -> p (s i) n", p=P)[
                        :, :, c0:c0 + w],
                ).then_inc(sem[ph], 16)

        def trickle_descs(eng, big, row0, subsems, ph):
            c0, w = COFF[ph], PHW[ph]
            for s in range(B_LOC):
                r = row0 + 3 * s
                eng.dma_start(
                    out=stg3[:, r:r + 3, c0:c0 + w],
                    in_=big[s].rearrange("i (p n) -> p i n", p=P)[:, :, c0:c0 + w],
                ).then_inc(subsems[s], 16)

        @block.sync
        def _(sync: bass.BassEngine):
            phase6(sync, sep, 0, stA, 0)
            sync.dma_start(
                out=p0.rearrange("p (s x) -> p s x", s=B_LOC),
                in_=diar.rearrange("s (p k) j -> p s (k j)", p=PD),
            ).then_inc(pdma_sem, 16)
            phase6(sync, sep, 0, stA, 1)
            phase6(sync, sep, 0, stA, 2)
            trickle_descs(sync, sep, 0, stA3, 3)
            trickle_descs(sync, sep, 0, stA4, 4)
            sync.wait_ge(dve_sem, 4)
            sync.dma_start(out=gram_out, in_=out_sb).then_inc(odma_sem, 16)
            sync.wait_ge(odma_sem, 16)

        @block.gpsimd
        def _(gpsimd: bass.BassEngine):
            gpsimd.memset(out_sb, 0.0).then_inc(gp_init, 1)
            ones_ap = blk.rearrange("p (g x) -> p g x", g=G_TOT)[:, :, 5 * NDATA:5 * NROW]
            gpsimd.memset(ones_ap, 1.0).then_inc(gp_init, 1)

        @block.scalar
        def _(scalar: bass.BassEngine):
            scalar.dma_start(
                out=lf.rearrange("p (s x) -> p s x", s=B_LOC),
                in_=lab.rearrange("s (p e) j -> p s (e j)", p=PD),
            ).then_inc(ldma_sem, 16)
            phase6(scalar, src, 12, stB, 0)

            rrk = rr.rearrange("p (k q s j) -> p k q s j", k=KSUB, q=2, s=B_LOC)
            llk = ll.rearrange("p (k q s j) -> p k q s j", k=KSUB, q=2, s=B_LOC)
            p0k = p0.rearrange("p (s k j) -> p k s j", s=B_LOC, k=KSUB)
            scalar.wait_ge(pdma_sem, 16)
            scalar.activation(llk[:, :, 0, :, :], p0k, AFT.Ln).then_inc(act_sem, 1)
            scalar.activation(llk[:, :, 1, :, :], p0k, AFT.Ln,
                              scale=-1.0, bias=1.0).then_inc(act_sem, 1)
            scalar.wait_ge(dve_sem, 1)
            scalar.activation(rrk[:, :, 1, :, :], rrk[:, :, 0, :, :], AFT.Copy,
                              scale=-1.0, bias=1.0).then_inc(act_sem, 1)

            phase6(scalar, src, 12, stB, 1)
            phase6(scalar, src, 12, stB, 2)
            trickle_descs(scalar, src, 12, stB3, 3)
            trickle_descs(scalar, src, 12, stB4, 4)

            # ACT tail casts: src rows 12-17 of ph3/ph4 (6-row chunks; DVE
            # takes the later-landing 18-23 chunks at its higher rate)
            def achunk(r0, subsems, i0, i1, ph):
                gl = slice(G0[ph], G0[ph] + GS[ph])
                scalar.wait_ge(subsems[i0], 16)
                scalar.wait_ge(subsems[i1], 16)
                scalar.activation(
                    blk4[:, r0:r0 + 6, gl, :],
                    stg4[:, r0:r0 + 6, gl, :],
                    AFT.Copy).then_inc(rpk_act, 1)

            achunk(12, stB3, 0, 1, 3)
            achunk(12, stB4, 0, 1, 4)

        @block.vector
        def _(vector: bass.BassEngine):
            def cast(r0, r1, g0, g1):
                vector.tensor_copy(
                    blk4[:, r0:r1, g0:g1, :],
                    stg4[:, r0:r1, g0:g1, :],
                ).then_inc(rpk_dve, 1)

            rrk = rr.rearrange("p (k q s j) -> p k q s j", k=KSUB, q=2, s=B_LOC)
            lf5 = lf.rearrange("p (s k f j) -> p k s f j", s=B_LOC, k=KSUB,
                               f=(TF // PD) // KSUB)[:, :, :, 0, :]
            vector.wait_ge(ldma_sem, 16)
            vector.tensor_copy(rrk[:, :, 0, :, :], lf5).then_inc(dve_sem, 1)
            vector.wait_ge(act_sem, 2)
            vector.tensor_scalar_max(ll[:, :], ll[:, :], -100.0).then_inc(dve_sem, 1)
            vector.wait_ge(gp_init, 1)
            vector.wait_ge(pe_sem, 1)
            vector.tensor_copy(out_sb[0:NDATA, PD:PD + NDATA], ps_d
                               ).then_inc(dve_sem, 1)
            for ph in (0, 1):
                vector.wait_ge(stA[ph], 32)
                vector.wait_ge(stB[ph], 32)
                cast(0, 24, G0[ph], G0[ph] + GS[ph])
            vector.wait_ge(stA[2], 32)
            vector.wait_ge(stB[2], 32)
            cast(0, 24, G0[2], G0[2] + H2)
            cast(0, 24, G0[2] + H2, G0[2] + GS[2])

            def vchunk(r0, subsems, i0, i1, ph):
                gl = (G0[ph], G0[ph] + GS[ph])
                vector.wait_ge(subsems[i0], 16)
                vector.wait_ge(subsems[i1], 16)
                cast(r0, r0 + 6, gl[0], gl[1])

            vchunk(0, stA3, 0, 1, 3)
            vchunk(6, stA3, 2, 3, 3)
            vchunk(18, stB3, 2, 3, 3)
            vchunk(0, stA4, 0, 1, 4)
            vchunk(6, stA4, 2, 3, 4)
            vchunk(18, stB4, 2, 3, 4)
            vector.wait_ge(pe_sem, 6)
            vector.tensor_copy(out_sb[:, 0:PD], ps_g[0:PD, :]).then_inc(dve_sem, 1)

        @block.tensor
        def _(tensor: bass.BassEngine):
            nmm = [0]

            def mms(g0, g1):
                mm = None
                for g in range(g0, g1):
                    lhsT = blk[:, BLK * g: BLK * g + 128]
                    rhs = blk[:, BLK * g: BLK * g + 125]
                    mm = tensor.matmul(ps_g, lhsT, rhs,
                                       start=(nmm[0] == 0), stop=(nmm[0] == G_TOT - 1))
                    nmm[0] += 1
                return mm

            tensor.wait_ge(act_sem, 3)
            tensor.wait_ge(dve_sem, 2)
            nd = 2 * B_LOC * C
            for k in range(KSUB):
                dmm = tensor.matmul(ps_d, ll[:, k * nd:(k + 1) * nd],
                                    rr[:, k * nd:(k + 1) * nd],
                                    start=(k == 0), stop=(k == KSUB - 1))
            dmm.then_inc(pe_sem, 1)

            tensor.wait_ge(gp_init, 2)
            tensor.wait_ge(rpk_dve, 1)
            mms(G0[0], G0[0] + GS[0]).then_inc(pe_sem, 1)
            tensor.wait_ge(rpk_dve, 2)
            mms(G0[1], G0[1] + GS[1]).then_inc(pe_sem, 1)
            tensor.wait_ge(rpk_dve, 3)
            mms(G0[2], G0[2] + H2)
            tensor.wait_ge(rpk_dve, 4)
            mms(G0[2] + H2, G0[2] + GS[2]).then_inc(pe_sem, 1)
            tensor.wait_ge(rpk_dve, 7)
            tensor.wait_ge(rpk_act, 1)
            mms(G0[3], G0[3] + GS[3]).then_inc(pe_sem, 1)
            tensor.wait_ge(rpk_dve, 10)
            tensor.wait_ge(rpk_act, 2)
            mms(G0[4], G0[4] + GS[4]).then_inc(pe_sem, 1)

    return nc, dict(T=T, PHW=PHW, tmajor=True)


def build_nc_v5(T=128000, PHW=(260, 240, 190, 180, 130)):
    """v5: desc-gen per dma_start is fixed-overhead dominated (~0.9us
    regardless of descriptor count), so phases 0-3 use 6-row DMAs (2 per
    phase per ring) — all descriptors generated early, the DGE queue's
    buffered backlog covers the stream tail. Diar inputs ride the ring
    FRONTS (p0 on A, lf on B) so the whole diar chain (Lns, complement,
    clamp, diar matmuls, ps_d copy) completes by ~15us. PE runs diar first,
    then gram phases with ph2/ph3 casts split into column halves. Last
    phase trickles per-sample as in v4."""
    TCOLS = T // P
    assert TCOLS * P == T
    assert sum(PHW) == TCOLS
    for w in PHW:
        assert w % 5 == 0 and w >= 128
    NPH = len(PHW)
    assert NPH == 5
    COFF = [sum(PHW[:i]) for i in range(NPH)]
    G0 = [c // 5 for c in COFF]
    GS = [w // 5 for w in PHW]
    G_TOT = TCOLS // 5
    TF = TSUB * 8
    LAST = NPH - 1
    H2 = GS[2] // 2
    H3 = GS[3] // 2

    nc = bass.Bass(trn_type="TRN2", target_bir_lowering=False, debug=False)

    sep = nc.dram_tensor("sep", [B_LOC, C, T], F32, kind="ExternalInput").ap()
    src = nc.dram_tensor("src", [B_LOC, C, T], F32, kind="ExternalInput").ap()
    diar = nc.dram_tensor("diar", [B_LOC, TSUB, C], F32, kind="ExternalInput").ap()
    lab = nc.dram_tensor("lab", [B_LOC, TF, C], F32, kind="ExternalInput").ap()
    gram_out = nc.dram_tensor("gram", [PD, PD + 27], F32, kind="ExternalOutput").ap()

    stg = nc.alloc_sbuf_tensor("stg", [P, NDATA * TCOLS], F32).ap()
    blk = nc.alloc_sbuf_tensor("blk", [P, NROW * TCOLS], BF16).ap()
    p0 = nc.alloc_sbuf_tensor("p0", [PD, B_LOC * KSUB * C], F32).ap()
    lf = nc.alloc_sbuf_tensor("lf", [PD, B_LOC * (TF // PD) * C], F32).ap()
    ll = nc.alloc_sbuf_tensor("ll", [PD, KSUB * 2 * B_LOC * C], F32).ap()
    rr = nc.alloc_sbuf_tensor("rr", [PD, KSUB * 2 * B_LOC * C], F32).ap()
    out_sb = nc.alloc_sbuf_tensor("out_sb", [PD, PD + 27], F32).ap()

    ps_g = nc.alloc_psum_tensor("ps_g", [P, PD], F32).ap()
    ps_d = nc.alloc_psum_tensor("ps_d", [NDATA, NDATA], F32).ap()

    stg3 = stg.rearrange("p (r n) -> p r n", r=NDATA)
    stg4 = stg.rearrange("p (r g c) -> p r g c", r=NDATA, g=G_TOT)
    blk4 = blk.rearrange("p (g r c) -> p r g c", g=G_TOT, r=NROW)

    with ExitStack() as ctx:
        stA = [ctx.enter_context(nc.semaphore(f"stA{i}")) for i in range(NPH)]
        stB = [ctx.enter_context(nc.semaphore(f"stB{i}")) for i in range(NPH)]
        stA4 = [ctx.enter_context(nc.semaphore(f"stA4_{i}")) for i in range(B_LOC)]
        stB4 = [ctx.enter_context(nc.semaphore(f"stB4_{i}")) for i in range(B_LOC)]
        pdma_sem = ctx.enter_context(nc.semaphore("pdma_sem"))
        ldma_sem = ctx.enter_context(nc.semaphore("ldma_sem"))
        odma_sem = ctx.enter_context(nc.semaphore("odma_sem"))
        gp_init = ctx.enter_context(nc.semaphore("gp_init"))
        act_sem = ctx.enter_context(nc.semaphore("act_sem"))
        dve_sem = ctx.enter_context(nc.semaphore("dve_sem"))
        pe_sem = ctx.enter_context(nc.semaphore("pe_sem"))
        rpk_dve = ctx.enter_context(nc.semaphore("rpk_dve"))
        rpk_act = ctx.enter_context(nc.semaphore("rpk_act"))
        block = ctx.enter_context(nc.Block())

        def phase6(eng, big, row0, sem, ph):
            """Two 6-row DMAs (samples 0-1, 2-3) for one phase."""
            c0, w = COFF[ph], PHW[ph]
            for s0 in (0, 2):
                r = row0 + 3 * s0
                eng.dma_start(
                    out=stg3[:, r:r + 6, c0:c0 + w],
                    in_=big[s0:s0 + 2].rearrange("s i (p n) -> p (s i) n", p=P)[
                        :, :, c0:c0 + w],
                ).then_inc(sem[ph], 16)

        def trickle_descs(eng, big, row0, subsems):
            c0, w = COFF[LAST], PHW[LAST]
            for s in range(B_LOC):
                r = row0 + 3 * s
                eng.dma_start(
                    out=stg3[:, r:r + 3, c0:c0 + w],
                    in_=big[s].rearrange("i (p n) -> p i n", p=P)[:, :, c0:c0 + w],
                ).then_inc(subsems[s], 16)

        @block.sync
        def _(sync: bass.BassEngine):
            sync.dma_start(
                out=p0.rearrange("p (s x) -> p s x", s=B_LOC),
                in_=diar.rearrange("s (p k) j -> p s (k j)", p=PD),
            ).then_inc(pdma_sem, 16)
            for ph in range(NPH - 1):
                phase6(sync, sep, 0, stA, ph)
            trickle_descs(sync, sep, 0, stA4)
            sync.wait_ge(dve_sem, 4)
            sync.dma_start(out=gram_out, in_=out_sb).then_inc(odma_sem, 16)
            sync.wait_ge(odma_sem, 16)

        @block.gpsimd
        def _(gpsimd: bass.BassEngine):
            gpsimd.memset(out_sb, 0.0).then_inc(gp_init, 1)
            ones_ap = blk.rearrange("p (g x) -> p g x", g=G_TOT)[:, :, 5 * NDATA:5 * NROW]
            gpsimd.memset(ones_ap, 1.0).then_inc(gp_init, 1)

        @block.scalar
        def _(scalar: bass.BassEngine):
            scalar.dma_start(
                out=lf.rearrange("p (s x) -> p s x", s=B_LOC),
                in_=lab.rearrange("s (p e) j -> p s (e j)", p=PD),
            ).then_inc(ldma_sem, 16)
            phase6(scalar, src, 12, stB, 0)

            rrk = rr.rearrange("p (k q s j) -> p k q s j", k=KSUB, q=2, s=B_LOC)
            llk = ll.rearrange("p (k q s j) -> p k q s j", k=KSUB, q=2, s=B_LOC)
            p0k = p0.rearrange("p (s k j) -> p k s j", s=B_LOC, k=KSUB)
            scalar.wait_ge(pdma_sem, 16)
            scalar.activation(llk[:, :, 0, :, :], p0k, AFT.Ln).then_inc(act_sem, 1)
            scalar.activation(llk[:, :, 1, :, :], p0k, AFT.Ln,
                              scale=-1.0, bias=1.0).then_inc(act_sem, 1)
            scalar.wait_ge(dve_sem, 1)
            scalar.activation(rrk[:, :, 1, :, :], rrk[:, :, 0, :, :], AFT.Copy,
                              scale=-1.0, bias=1.0).then_inc(act_sem, 1)

            for ph in range(1, NPH - 1):
                phase6(scalar, src, 12, stB, ph)
            trickle_descs(scalar, src, 12, stB4)
            gl = slice(G0[LAST], G0[LAST] + GS[LAST])
            for i in range(4):
                r = 12 + 3 * i
                scalar.wait_ge(stB4[i], 16)
                scalar.activation(
                    blk4[:, r:r + 3, gl, :],
                    stg4[:, r:r + 3, gl, :],
                    AFT.Copy).then_inc(rpk_act, 1)

        @block.vector
        def _(vector: bass.BassEngine):
            def cast(r0, r1, g0, g1):
                vector.tensor_copy(
                    blk4[:, r0:r1, g0:g1, :],
                    stg4[:, r0:r1, g0:g1, :],
                ).then_inc(rpk_dve, 1)

            rrk = rr.rearrange("p (k q s j) -> p k q s j", k=KSUB, q=2, s=B_LOC)
            lf5 = lf.rearrange("p (s k f j) -> p k s f j", s=B_LOC, k=KSUB,
                               f=(TF // PD) // KSUB)[:, :, :, 0, :]
            vector.wait_ge(ldma_sem, 16)
            vector.tensor_copy(rrk[:, :, 0, :, :], lf5).then_inc(dve_sem, 1)
            vector.wait_ge(act_sem, 2)
            vector.tensor_scalar_max(ll[:, :], ll[:, :], -100.0).then_inc(dve_sem, 1)
            vector.wait_ge(gp_init, 1)
            vector.wait_ge(pe_sem, 1)
            vector.tensor_copy(out_sb[0:NDATA, PD:PD + NDATA], ps_d
                               ).then_inc(dve_sem, 1)
            for ph in (0, 1):
                vector.wait_ge(stA[ph], 32)
                vector.wait_ge(stB[ph], 32)
                cast(0, 24, G0[ph], G0[ph] + GS[ph])
            vector.wait_ge(stA[2], 32)
            vector.wait_ge(stB[2], 32)
            cast(0, 24, G0[2], G0[2] + H2)
            cast(0, 24, G0[2] + H2, G0[2] + GS[2])
            vector.wait_ge(stA[3], 32)
            vector.wait_ge(stB[3], 32)
            cast(0, 24, G0[3], G0[3] + H3)
            cast(0, 24, G0[3] + H3, G0[3] + GS[3])
            gl4 = (G0[LAST], G0[LAST] + GS[LAST])
            for i in range(4):
                vector.wait_ge(stA4[i], 16)
                cast(3 * i, 3 * i + 3, gl4[0], gl4[1])
            vector.wait_ge(pe_sem, 6)
            vector.tensor_copy(out_sb[:, 0:PD], ps_g[0:PD, :]).then_inc(dve_sem, 1)

        @block.tensor
        def _(tensor: bass.BassEngine):
            nmm = [0]

            def mms(g0, g1):
                mm = None
                for g in range(g0, g1):
                    lhsT = blk[:, BLK * g: BLK * g + 128]
                    rhs = blk[:, BLK * g: BLK * g + 125]
                    mm = tensor.matmul(ps_g, lhsT, rhs,
                                       start=(nmm[0] == 0), stop=(nmm[0] == G_TOT - 1))
                    nmm[0] += 1
                return mm

            tensor.wait_ge(act_sem, 3)
            tensor.wait_ge(dve_sem, 2)
            nd = 2 * B_LOC * C
            for k in range(KSUB):
                dmm = tensor.matmul(ps_d, ll[:, k * nd:(k + 1) * nd],
                                    rr[:, k * nd:(k + 1) * nd],
                                    start=(k == 0), stop=(k == KSUB - 1))
            dmm.then_inc(pe_sem, 1)

            tensor.wait_ge(gp_init, 2)
            tensor.wait_ge(rpk_dve, 1)
            mms(G0[0], G0[0] + GS[0]).then_inc(pe_sem, 1)
            tensor.wait_ge(rpk_dve, 2)
            mms(G0[1], G0[1] + GS[1]).then_inc(pe_sem, 1)
            tensor.wait_ge(rpk_dve, 3)
            mms(G0[2], G0[2] + H2)
            tensor.wait_ge(rpk_dve, 4)
            mms(G0[2] + H2, G0[2] + GS[2]).then_inc(pe_sem, 1)
            tensor.wait_ge(rpk_dve, 5)
            mms(G0[3], G0[3] + H3)
            tensor.wait_ge(rpk_dve, 6)
            mms(G0[3] + H3, G0[3] + GS[3]).then_inc(pe_sem, 1)
            tensor.wait_ge(rpk_dve, 10)
            tensor.wait_ge(rpk_act, 4)
            mms(G0[LAST], G0[LAST] + GS[LAST]).then_inc(pe_sem, 1)

    return nc, dict(T=T, PHW=PHW, tmajor=True)


def build_nc_v4(T=128000, PHW=(260, 240, 190, 180, 130), chunk6=False,
                no_gp_drain=False, diar_front=False):
    """v4: 3-row DMAs (desc-gen flow-controls against queue depth without
    hogging), DVE does ALL repack casts (measured ~197 G elem/s), scalar
    only ring-B descs + diar chain + last-phase trickle casts, mid phases'
    casts split in column halves so PE starts earlier, diar input DMAs
    tucked mid-ring (p0 on A, lf on B), diar matmuls between gram ph2/ph3.
    Rows t-major as v3. chunk6: cast the trickled last phase in 6-row
    chunks (halves per-instruction overhead on the tail)."""
    TCOLS = T // P
    assert TCOLS * P == T
    assert sum(PHW) == TCOLS
    for w in PHW:
        assert w % 5 == 0 and w >= 128
    NPH = len(PHW)
    assert NPH == 5
    COFF = [sum(PHW[:i]) for i in range(NPH)]
    G0 = [c // 5 for c in COFF]
    GS = [w // 5 for w in PHW]
    G_TOT = TCOLS // 5
    TF = TSUB * 8
    LAST = NPH - 1
    # column-half splits for ph2/ph3 (block counts)
    H2 = GS[2] // 2
    H3 = GS[3] // 2

    nc = bass.Bass(trn_type="TRN2", target_bir_lowering=False, debug=False)

    sep = nc.dram_tensor("sep", [B_LOC, C, T], F32, kind="ExternalInput").ap()
    src = nc.dram_tensor("src", [B_LOC, C, T], F32, kind="ExternalInput").ap()
    diar = nc.dram_tensor("diar", [B_LOC, TSUB, C], F32, kind="ExternalInput").ap()
    lab = nc.dram_tensor("lab", [B_LOC, TF, C], F32, kind="ExternalInput").ap()
    gram_out = nc.dram_tensor("gram", [PD, PD + 27], F32, kind="ExternalOutput").ap()

    stg = nc.alloc_sbuf_tensor("stg", [P, NDATA * TCOLS], F32).ap()
    blk = nc.alloc_sbuf_tensor("blk", [P, NROW * TCOLS], BF16).ap()
    p0 = nc.alloc_sbuf_tensor("p0", [PD, B_LOC * KSUB * C], F32).ap()
    lf = nc.alloc_sbuf_tensor("lf", [PD, B_LOC * (TF // PD) * C], F32).ap()
    ll = nc.alloc_sbuf_tensor("ll", [PD, KSUB * 2 * B_LOC * C], F32).ap()
    rr = nc.alloc_sbuf_tensor("rr", [PD, KSUB * 2 * B_LOC * C], F32).ap()
    out_sb = nc.alloc_sbuf_tensor("out_sb", [PD, PD + 27], F32).ap()

    ps_g = nc.alloc_psum_tensor("ps_g", [P, PD], F32).ap()
    ps_d = nc.alloc_psum_tensor("ps_d", [NDATA, NDATA], F32).ap()

    stg3 = stg.rearrange("p (r n) -> p r n", r=NDATA)
    stg4 = stg.rearrange("p (r g c) -> p r g c", r=NDATA, g=G_TOT)
    blk4 = blk.rearrange("p (g r c) -> p r g c", g=G_TOT, r=NROW)

    with ExitStack() as ctx:
        stA = [ctx.enter_context(nc.semaphore(f"stA{i}")) for i in range(NPH)]
        stB = [ctx.enter_context(nc.semaphore(f"stB{i}")) for i in range(NPH)]
        # per-sample sems for the trickled last phase: parallel DMA engines can
        # complete the 4 sub-DMAs slightly out of ring order, so a cumulative
        # count can fire before the specific sample's data is resident
        stA4 = [ctx.enter_context(nc.semaphore(f"stA4_{i}")) for i in range(B_LOC)]
        stB4 = [ctx.enter_context(nc.semaphore(f"stB4_{i}")) for i in range(B_LOC)]
        pdma_sem = ctx.enter_context(nc.semaphore("pdma_sem"))
        ldma_sem = ctx.enter_context(nc.semaphore("ldma_sem"))
        odma_sem = ctx.enter_context(nc.semaphore("odma_sem"))
        gp_init = ctx.enter_context(nc.semaphore("gp_init"))
        act_sem = ctx.enter_context(nc.semaphore("act_sem"))
        dve_sem = ctx.enter_context(nc.semaphore("dve_sem"))
        pe_sem = ctx.enter_context(nc.semaphore("pe_sem"))
        rpk_dve = ctx.enter_context(nc.semaphore("rpk_dve"))
        rpk_act = ctx.enter_context(nc.semaphore("rpk_act"))
        block = ctx.enter_context(nc.Block(no_gpsimd_drain=no_gp_drain))

        def phase_descs(eng, big, row0, sem, ph, subsems=None):
            c0, w = COFF[ph], PHW[ph]
            for s in range(B_LOC):
                r = row0 + 3 * s
                dma = eng.dma_start(
                    out=stg3[:, r:r + 3, c0:c0 + w],
                    in_=big[s].rearrange("i (p n) -> p i n", p=P)[:, :, c0:c0 + w],
                )
                dma.then_inc(subsems[s] if subsems is not None else sem[ph], 16)

        @block.sync
        def _(sync: bass.BassEngine):
            def p0_desc():
                sync.dma_start(
                    out=p0.rearrange("p (s x) -> p s x", s=B_LOC),
                    in_=diar.rearrange("s (p k) j -> p s (k j)", p=PD),
                ).then_inc(pdma_sem, 16)

            if diar_front:
                p0_desc()
            phase_descs(sync, sep, 0, stA, 0)
            phase_descs(sync, sep, 0, stA, 1)
            if not diar_front:
                p0_desc()
            phase_descs(sync, sep, 0, stA, 2)
            phase_descs(sync, sep, 0, stA, 3)
            phase_descs(sync, sep, 0, stA, LAST, subsems=stA4)
            sync.wait_ge(dve_sem, 4)
            sync.dma_start(out=gram_out, in_=out_sb).then_inc(odma_sem, 16)
            sync.wait_ge(odma_sem, 16)

        @block.gpsimd
        def _(gpsimd: bass.BassEngine):
            gpsimd.memset(out_sb, 0.0).then_inc(gp_init, 1)
            ones_ap = blk.rearrange("p (g x) -> p g x", g=G_TOT)[:, :, 5 * NDATA:5 * NROW]
            gpsimd.memset(ones_ap, 1.0).then_inc(gp_init, 1)

        @block.scalar
        def _(scalar: bass.BassEngine):
            rrk = rr.rearrange("p (k q s j) -> p k q s j", k=KSUB, q=2, s=B_LOC)
            llk = ll.rearrange("p (k q s j) -> p k q s j", k=KSUB, q=2, s=B_LOC)
            p0k = p0.rearrange("p (s k j) -> p k s j", s=B_LOC, k=KSUB)

            def lf_desc():
                scalar.dma_start(
                    out=lf.rearrange("p (s x) -> p s x", s=B_LOC),
                    in_=lab.rearrange("s (p e) j -> p s (e j)", p=PD),
                ).then_inc(ldma_sem, 16)

            def dance():
                scalar.wait_ge(pdma_sem, 16)
                scalar.activation(llk[:, :, 0, :, :], p0k, AFT.Ln
                                  ).then_inc(act_sem, 1)
                scalar.activation(llk[:, :, 1, :, :], p0k, AFT.Ln,
                                  scale=-1.0, bias=1.0).then_inc(act_sem, 1)
                scalar.wait_ge(dve_sem, 1)
                scalar.activation(rrk[:, :, 1, :, :], rrk[:, :, 0, :, :],
                                  AFT.Copy, scale=-1.0, bias=1.0
                                  ).then_inc(act_sem, 1)

            if diar_front:
                lf_desc()
            phase_descs(scalar, src, 12, stB, 0)
            phase_descs(scalar, src, 12, stB, 1)
            if not diar_front:
                lf_desc()
            if diar_front:
                dance()
            phase_descs(scalar, src, 12, stB, 2)
            phase_descs(scalar, src, 12, stB, 3)
            if not diar_front:
                dance()

            phase_descs(scalar, src, 12, stB, LAST, subsems=stB4)
            # trickle-cast ph4 src rows as their DMAs land
            gl = slice(G0[LAST], G0[LAST] + GS[LAST])
            if chunk6:
                for j in (0, 1):
                    r = 12 + 6 * j
                    scalar.wait_ge(stB4[2 * j], 16)
                    scalar.wait_ge(stB4[2 * j + 1], 16)
                    scalar.activation(
                        blk4[:, r:r + 6, gl, :],
                        stg4[:, r:r + 6, gl, :],
                        AFT.Copy).then_inc(rpk_act, 1)
            else:
                for i in range(4):
                    r = 12 + 3 * i
                    scalar.wait_ge(stB4[i], 16)
                    scalar.activation(
                        blk4[:, r:r + 3, gl, :],
                        stg4[:, r:r + 3, gl, :],
                        AFT.Copy).then_inc(rpk_act, 1)

        @block.vector
        def _(vector: bass.BassEngine):
            def cast(r0, r1, g0, g1):
                vector.tensor_copy(
                    blk4[:, r0:r1, g0:g1, :],
                    stg4[:, r0:r1, g0:g1, :],
                ).then_inc(rpk_dve, 1)

            for ph in (0, 1):
                vector.wait_ge(stA[ph], 64)
                vector.wait_ge(stB[ph], 64)
                cast(0, 24, G0[ph], G0[ph] + GS[ph])
            rrk = rr.rearrange("p (k q s j) -> p k q s j", k=KSUB, q=2, s=B_LOC)
            lf5 = lf.rearrange("p (s k f j) -> p k s f j", s=B_LOC, k=KSUB,
                               f=(TF // PD) // KSUB)[:, :, :, 0, :]
            vector.wait_ge(ldma_sem, 16)
            vector.tensor_copy(rrk[:, :, 0, :, :], lf5).then_inc(dve_sem, 1)
            vector.wait_ge(act_sem, 2)
            vector.tensor_scalar_max(ll[:, :], ll[:, :], -100.0).then_inc(dve_sem, 1)
            vector.wait_ge(stA[2], 64)
            vector.wait_ge(stB[2], 64)
            cast(0, 24, G0[2], G0[2] + H2)
            cast(0, 24, G0[2] + H2, G0[2] + GS[2])
            vector.wait_ge(stA[3], 64)
            vector.wait_ge(stB[3], 64)
            cast(0, 24, G0[3], G0[3] + H3)
            cast(0, 24, G0[3] + H3, G0[3] + GS[3])
            vector.wait_ge(gp_init, 1)
            vector.wait_ge(pe_sem, 4)
            vector.tensor_copy(out_sb[0:NDATA, PD:PD + NDATA], ps_d
                               ).then_inc(dve_sem, 1)
            # trickle-cast ph4 sep rows as their DMAs land
            gl4 = (G0[LAST], G0[LAST] + GS[LAST])
            if chunk6:
                for j in (0, 1):
                    vector.wait_ge(stA4[2 * j], 16)
                    vector.wait_ge(stA4[2 * j + 1], 16)
                    cast(6 * j, 6 * j + 6, gl4[0], gl4[1])
            else:
                for i in range(4):
                    vector.wait_ge(stA4[i], 16)
                    cast(3 * i, 3 * i + 3, gl4[0], gl4[1])
            vector.wait_ge(pe_sem, 6)
            vector.tensor_copy(out_sb[:, 0:PD], ps_g[0:PD, :]).then_inc(dve_sem, 1)

        @block.tensor
        def _(tensor: bass.BassEngine):
            nmm = [0]

            def mms(g0, g1):
                mm = None
                for g in range(g0, g1):
                    lhsT = blk[:, BLK * g: BLK * g + 128]
                    rhs = blk[:, BLK * g: BLK * g + 125]
                    mm = tensor.matmul(ps_g, lhsT, rhs,
                                       start=(nmm[0] == 0), stop=(nmm[0] == G_TOT - 1))
                    nmm[0] += 1
                return mm

            tensor.wait_ge(gp_init, 2)
            tensor.wait_ge(rpk_dve, 1)
            mms(G0[0], G0[0] + GS[0]).then_inc(pe_sem, 1)
            tensor.wait_ge(rpk_dve, 2)
            mms(G0[1], G0[1] + GS[1]).then_inc(pe_sem, 1)
            tensor.wait_ge(rpk_dve, 3)
            mms(G0[2], G0[2] + H2)
            tensor.wait_ge(rpk_dve, 4)
            mms(G0[2] + H2, G0[2] + GS[2]).then_inc(pe_sem, 1)

            tensor.wait_ge(act_sem, 3)
            tensor.wait_ge(dve_sem, 2)
            nd = 2 * B_LOC * C
            for k in range(KSUB):
                dmm = tensor.matmul(ps_d, ll[:, k * nd:(k + 1) * nd],
                                    rr[:, k * nd:(k + 1) * nd],
                                    start=(k == 0), stop=(k == KSUB - 1))
            dmm.then_inc(pe_sem, 1)

            tensor.wait_ge(rpk_dve, 5)
            mms(G0[3], G0[3] + H3)
            tensor.wait_ge(rpk_dve, 6)
            mms(G0[3] + H3, G0[3] + GS[3]).then_inc(pe_sem, 1)
            tensor.wait_ge(rpk_dve, 8 if chunk6 else 10)
            tensor.wait_ge(rpk_act, 2 if chunk6 else 4)
            mms(G0[LAST], G0[LAST] + GS[LAST]).then_inc(pe_sem, 1)

    return nc, dict(T=T, PHW=PHW, tmajor=True)


def build_nc_v3(T=128000, PHW=(260, 240, 190, 180, 130)):
    """v3: rows t-major (sep rows 0-11, src rows 12-23). Ring A (sync) DMAs
    all sep rows in one 12-row DMA per phase; ring B (scalar) all src rows.
    Last phase split into per-sample DMAs so repack trickles 3 rows at a
    time right behind the drain. Diar input DMAs ride the gpsimd SWDGE
    queue (drains early, keeps both HWDGE rings pure). lhsT is 128-wide to
    trigger FWL. Host must map e_rows=s*3+i, t_rows=12+s*3+j (tmajor)."""
    TCOLS = T // P
    assert TCOLS * P == T
    assert sum(PHW) == TCOLS
    for w in PHW:
        assert w % 5 == 0 and w >= 128
    NPH = len(PHW)
    COFF = [sum(PHW[:i]) for i in range(NPH)]
    G0 = [c // 5 for c in COFF]
    GS = [w // 5 for w in PHW]
    G_TOT = TCOLS // 5
    TF = TSUB * 8
    LAST = NPH - 1

    # cumulative repack-sem targets per phase (last phase trickles)
    DVE_CHUNKS = [1] * (NPH - 1) + [5]   # rows 0-14: 12 from A (4 chunks) + 3 from B
    ACT_CHUNKS = [1] * (NPH - 1) + [3]   # rows 15-23 from B
    CUM_DVE = [sum(DVE_CHUNKS[:i + 1]) for i in range(NPH)]
    CUM_ACT = [sum(ACT_CHUNKS[:i + 1]) for i in range(NPH)]

    nc = bass.Bass(trn_type="TRN2", target_bir_lowering=False, debug=False)

    sep = nc.dram_tensor("sep", [B_LOC, C, T], F32, kind="ExternalInput").ap()
    src = nc.dram_tensor("src", [B_LOC, C, T], F32, kind="ExternalInput").ap()
    diar = nc.dram_tensor("diar", [B_LOC, TSUB, C], F32, kind="ExternalInput").ap()
    lab = nc.dram_tensor("lab", [B_LOC, TF, C], F32, kind="ExternalInput").ap()
    gram_out = nc.dram_tensor("gram", [PD, PD + 27], F32, kind="ExternalOutput").ap()

    stg = nc.alloc_sbuf_tensor("stg", [P, NDATA * TCOLS], F32).ap()
    blk = nc.alloc_sbuf_tensor("blk", [P, NROW * TCOLS], BF16).ap()
    p0 = nc.alloc_sbuf_tensor("p0", [PD, B_LOC * KSUB * C], F32).ap()
    lf = nc.alloc_sbuf_tensor("lf", [PD, B_LOC * (TF // PD) * C], F32).ap()
    ll = nc.alloc_sbuf_tensor("ll", [PD, KSUB * 2 * B_LOC * C], F32).ap()
    rr = nc.alloc_sbuf_tensor("rr", [PD, KSUB * 2 * B_LOC * C], F32).ap()
    out_sb = nc.alloc_sbuf_tensor("out_sb", [PD, PD + 27], F32).ap()

    ps_g = nc.alloc_psum_tensor("ps_g", [P, PD], F32).ap()   # 128 rows for FWL
    ps_d = nc.alloc_psum_tensor("ps_d", [NDATA, NDATA], F32).ap()

    stg3 = stg.rearrange("p (r n) -> p r n", r=NDATA)
    stg4 = stg.rearrange("p (r g c) -> p r g c", r=NDATA, g=G_TOT)
    blk4 = blk.rearrange("p (g r c) -> p r g c", g=G_TOT, r=NROW)

    with ExitStack() as ctx:
        stA = [ctx.enter_context(nc.semaphore(f"stA{i}")) for i in range(NPH)]
        stB = [ctx.enter_context(nc.semaphore(f"stB{i}")) for i in range(NPH)]
        pdma_sem = ctx.enter_context(nc.semaphore("pdma_sem"))
        ldma_sem = ctx.enter_context(nc.semaphore("ldma_sem"))
        odma_sem = ctx.enter_context(nc.semaphore("odma_sem"))
        gp_init = ctx.enter_context(nc.semaphore("gp_init"))
        act_sem = ctx.enter_context(nc.semaphore("act_sem"))
        dve_sem = ctx.enter_context(nc.semaphore("dve_sem"))
        pe_sem = ctx.enter_context(nc.semaphore("pe_sem"))
        rpk_dve = ctx.enter_context(nc.semaphore("rpk_dve"))
        rpk_act = ctx.enter_context(nc.semaphore("rpk_act"))
        block = ctx.enter_context(nc.Block())

        def issue_ring(eng, big, row0, sem):
            """One 12-row DMA per phase for ph0..ph(N-2); last phase split
            into per-sample 3-row DMAs so repack can trickle."""
            all12 = big.rearrange("s i (p n) -> p (s i) n", p=P)
            for ph in range(NPH - 1):
                c0, w = COFF[ph], PHW[ph]
                eng.dma_start(out=stg3[:, row0:row0 + 12, c0:c0 + w],
                              in_=all12[:, :, c0:c0 + w]).then_inc(sem[ph], 16)
            c0, w = COFF[LAST], PHW[LAST]
            for s in range(B_LOC):
                r = row0 + 3 * s
                eng.dma_start(
                    out=stg3[:, r:r + 3, c0:c0 + w],
                    in_=big[s].rearrange("i (p n) -> p i n", p=P)[:, :, c0:c0 + w],
                ).then_inc(sem[LAST], 16)

        @block.sync
        def _(sync: bass.BassEngine):
            issue_ring(sync, sep, 0, stA)
            sync.wait_ge(dve_sem, 4)
            sync.dma_start(out=gram_out, in_=out_sb).then_inc(odma_sem, 16)
            sync.wait_ge(odma_sem, 16)

        @block.gpsimd
        def _(gpsimd: bass.BassEngine):
            gpsimd.dma_start(
                out=p0.rearrange("p (s x) -> p s x", s=B_LOC),
                in_=diar.rearrange("s (p k) j -> p s (k j)", p=PD),
            ).then_inc(pdma_sem, 16)
            gpsimd.dma_start(
                out=lf.rearrange("p (s x) -> p s x", s=B_LOC),
                in_=lab.rearrange("s (p e) j -> p s (e j)", p=PD),
            ).then_inc(ldma_sem, 16)
            gpsimd.memset(out_sb, 0.0).then_inc(gp_init, 1)
            ones_ap = blk.rearrange("p (g x) -> p g x", g=G_TOT)[:, :, 5 * NDATA:5 * NROW]
            gpsimd.memset(ones_ap, 1.0).then_inc(gp_init, 1)

        @block.scalar
        def _(scalar: bass.BassEngine):
            all12 = src.rearrange("s i (p n) -> p (s i) n", p=P)

            def b_desc(ph):
                c0, w = COFF[ph], PHW[ph]
                scalar.dma_start(out=stg3[:, 12:24, c0:c0 + w],
                                 in_=all12[:, :, c0:c0 + w]).then_inc(stB[ph], 16)

            b_desc(0)
            b_desc(1)
            # diar chain while ring-B descs are 2 phases ahead
            rrk = rr.rearrange("p (k q s j) -> p k q s j", k=KSUB, q=2, s=B_LOC)
            llk = ll.rearrange("p (k q s j) -> p k q s j", k=KSUB, q=2, s=B_LOC)
            p0k = p0.rearrange("p (s k j) -> p k s j", s=B_LOC, k=KSUB)
            scalar.wait_ge(pdma_sem, 16)
            scalar.activation(llk[:, :, 0, :, :], p0k, AFT.Ln).then_inc(act_sem, 1)
            scalar.activation(llk[:, :, 1, :, :], p0k, AFT.Ln,
                              scale=-1.0, bias=1.0).then_inc(act_sem, 1)
            scalar.wait_ge(dve_sem, 1)
            scalar.activation(rrk[:, :, 1, :, :], rrk[:, :, 0, :, :], AFT.Copy,
                              scale=-1.0, bias=1.0).then_inc(act_sem, 1)
            b_desc(2)
            b_desc(3)
            c0, w = COFF[LAST], PHW[LAST]
            for s in range(B_LOC):
                r = 12 + 3 * s
                scalar.dma_start(
                    out=stg3[:, r:r + 3, c0:c0 + w],
                    in_=src[s].rearrange("i (p n) -> p i n", p=P)[:, :, c0:c0 + w],
                ).then_inc(stB[LAST], 16)

            # repack rows 15-23 (ring B)
            for ph in range(NPH - 1):
                scalar.wait_ge(stB[ph], 16)
                scalar.activation(
                    blk4[:, 15:24, G0[ph]:G0[ph] + GS[ph], :],
                    stg4[:, 15:24, G0[ph]:G0[ph] + GS[ph], :],
                    AFT.Copy).then_inc(rpk_act, 1)
            gl = slice(G0[LAST], G0[LAST] + GS[LAST])
            for i, r in enumerate((15, 18, 21)):
                scalar.wait_ge(stB[LAST], 32 + 16 * i)
                scalar.activation(
                    blk4[:, r:r + 3, gl, :],
                    stg4[:, r:r + 3, gl, :],
                    AFT.Copy).then_inc(rpk_act, 1)

        @block.vector
        def _(vector: bass.BassEngine):
            rrk = rr.rearrange("p (k q s j) -> p k q s j", k=KSUB, q=2, s=B_LOC)
            lf5 = lf.rearrange("p (s k f j) -> p k s f j", s=B_LOC, k=KSUB,
                               f=(TF // PD) // KSUB)[:, :, :, 0, :]
            vector.wait_ge(ldma_sem, 16)
            vector.tensor_copy(rrk[:, :, 0, :, :], lf5).then_inc(dve_sem, 1)
            vector.wait_ge(act_sem, 2)
            vector.tensor_scalar_max(ll[:, :], ll[:, :], -100.0).then_inc(dve_sem, 1)
            vector.wait_ge(gp_init, 1)
            vector.wait_ge(pe_sem, 1)
            vector.tensor_copy(out_sb[0:NDATA, PD:PD + NDATA], ps_d
                               ).then_inc(dve_sem, 1)
            # repack rows 0-14 (12 from ring A + 3 from ring B)
            for ph in range(NPH - 1):
                vector.wait_ge(stA[ph], 16)
                vector.wait_ge(stB[ph], 16)
                vector.tensor_copy(
                    blk4[:, 0:15, G0[ph]:G0[ph] + GS[ph], :],
                    stg4[:, 0:15, G0[ph]:G0[ph] + GS[ph], :],
                ).then_inc(rpk_dve, 1)
            gl = slice(G0[LAST], G0[LAST] + GS[LAST])
            for i in range(4):
                vector.wait_ge(stA[LAST], 16 * (i + 1))
                vector.tensor_copy(
                    blk4[:, 3 * i:3 * i + 3, gl, :],
                    stg4[:, 3 * i:3 * i + 3, gl, :],
                ).then_inc(rpk_dve, 1)
            vector.wait_ge(stB[LAST], 16)
            vector.tensor_copy(
                blk4[:, 12:15, gl, :],
                stg4[:, 12:15, gl, :],
            ).then_inc(rpk_dve, 1)
            vector.wait_ge(pe_sem, NPH + 1)
            vector.tensor_copy(out_sb[:, 0:PD], ps_g[0:PD, :]).then_inc(dve_sem, 1)

        @block.tensor
        def _(tensor: bass.BassEngine):
            tensor.wait_ge(act_sem, 3)
            tensor.wait_ge(dve_sem, 2)
            nd = 2 * B_LOC * C
            for k in range(KSUB):
                dmm = tensor.matmul(ps_d, ll[:, k * nd:(k + 1) * nd],
                                    rr[:, k * nd:(k + 1) * nd],
                                    start=(k == 0), stop=(k == KSUB - 1))
            dmm.then_inc(pe_sem, 1)

            tensor.wait_ge(gp_init, 2)
            nmm = 0
            for ph in range(NPH):
                tensor.wait_ge(rpk_dve, CUM_DVE[ph])
                tensor.wait_ge(rpk_act, CUM_ACT[ph])
                for g in range(G0[ph], G0[ph] + GS[ph]):
                    lhsT = blk[:, BLK * g: BLK * g + 128]   # 128-wide: FWL
                    rhs = blk[:, BLK * g: BLK * g + 125]
                    mm = tensor.matmul(ps_g, lhsT, rhs,
                                       start=(nmm == 0), stop=(nmm == G_TOT - 1))
                    nmm += 1
                mm.then_inc(pe_sem, 1)

    return nc, dict(T=T, PHW=PHW, tmajor=True)


# ---------------- host side ----------------

EPS = 1e-8
LAM_SISNR, LAM_DIAR, LAM_EXIST = 1.0, 0.2, 0.2
from itertools import permutations
PERMS = np.array(list(permutations(range(C))), dtype=np.int64)  # [6, 3]


def host_gamma_fp32(g125):
    """g125 [125,125] -> Gamma [25,25]; m = 5*r + c."""
    return np.einsum('acbc->ab', g125.reshape(25, 5, 25, 5).astype(np.float64))


def host_gamma_f32r(ga, gb):
    """ga/gb [125,260] -> Gamma [25,25]; m = 5*ra + ca, n = rb*10 + 5*h + cb."""
    a = ga.reshape(25, 5, 26, 2, 5).astype(np.float64)   # [ra, ca, rb, h, cb]
    b = gb.reshape(25, 5, 26, 2, 5).astype(np.float64)
    gam = np.zeros((25, 25), np.float64)
    for cc in range(5):
        gam += a[:, cc, 0:25, 0, cc]
        gam += b[:, cc, 0:25, 1, cc]
    return gam


def _clog(x):
    with np.errstate(divide='ignore'):
        return np.maximum(np.log(x), -100.0)


def host_finalize(gammas, dgrams, exist_probs, num_speakers, T=128000,
                  tmajor=False):
    """gammas: list of [25,25] float64 per core; dgrams list of [24,24].
    tmajor: data rows are t*12 + s*3 + i (v3 layout) instead of s*6 + t*3 + i.
    Returns the 5 scalars (np.float32)."""
    B = len(gammas) * B_LOC
    ns = np.asarray(num_speakers).astype(np.int64)

    S = np.zeros((B, C, C), np.float64)
    D = np.zeros((B, C, C), np.float64)
    for core, (gam, dg) in enumerate(zip(gammas, dgrams)):
        dg = dg.astype(np.float64)
        for s in range(B_LOC):
            b = core * B_LOC + s
            if tmajor:
                e_rows = [s * 3 + i for i in range(3)]
                t_rows = [12 + s * 3 + j for j in range(3)]
            else:
                e_rows = [s * 6 + i for i in range(3)]
                t_rows = [s * 6 + 3 + j for j in range(3)]
            dot_raw = gam[np.ix_(e_rows, t_rows)]            # [i, j]
            sep_sq = np.array([gam[r, r] for r in e_rows])
            src_sq = np.array([gam[r, r] for r in t_rows])
            sum_sep = gam[e_rows, 24]
            sum_src = gam[t_rows, 24]

            dot = dot_raw - np.outer(sum_sep, sum_src) / T
            est_sq = sep_sq - sum_sep ** 2 / T               # [i]
            tgt_sq = src_sq - sum_src ** 2 / T               # [j]

            alpha = dot / (tgt_sq[None, :] + EPS)
            sig = alpha * alpha * tgt_sq[None, :] + EPS
            noise = est_sq[:, None] - 2.0 * alpha * dot + alpha * alpha * tgt_sq[None, :] + EPS
            S[b] = 10.0 * np.log10(sig / noise)

            A = dg[s * 3:s * 3 + 3, s * 3:s * 3 + 3]
            Bm = dg[12 + s * 3:12 + s * 3 + 3, 12 + s * 3:12 + s * 3 + 3]
            D[b] = -(A + Bm) / TSUB

    n_spk = np.clip(ns, 1, C)
    slot = np.arange(C)
    slot_mask = (slot[None, :] < n_spk[:, None]).astype(np.float64)
    valid = np.all((PERMS[None, :, :] < n_spk[:, None, None])
                   | (slot[None, None, :] >= n_spk[:, None, None]), axis=-1)

    S_perm = S[:, PERMS, slot]                               # [B, 6, 3]
    sisnr_mean = (S_perm * slot_mask[:, None, :]).sum(-1) / n_spk[:, None]
    sisnr_loss_p = np.where(valid, -sisnr_mean, np.inf)
    best = sisnr_loss_p.min(axis=-1)
    loss_sisnr = best.mean()
    mean_sisnr = (-best).mean()

    D_perm = D[:, PERMS, slot]
    diar_p = (D_perm * slot_mask[:, None, :]).sum(-1) / n_spk[:, None]
    loss_diar = np.where(valid, diar_p, np.inf).min(axis=-1).mean()

    ep = np.asarray(exist_probs, np.float64)
    n_ex = np.minimum(ns, C)
    ex_tgt = (np.arange(C + 1)[None, :] < n_ex[:, None]).astype(np.float64)
    bce_ex = -(ex_tgt * _clog(ep) + (1.0 - ex_tgt) * _clog(1.0 - ep))
    loss_exist = bce_ex.mean()

    total = LAM_SISNR * loss_sisnr + LAM_DIAR * loss_diar + LAM_EXIST * loss_exist
    return tuple(np.float32(v) for v in
                 (total, loss_sisnr, loss_diar, loss_exist, mean_sisnr))


def shard_inputs(separated, diar_probs, sources, labels, n_cores=8):
    maps = []
    for c in range(n_cores):
        sl = slice(B_LOC * c, B_LOC * (c + 1))
        maps.append({
            "sep": np.ascontiguousarray(separated[sl], dtype=np.float32),
            "src": np.ascontiguousarray(sources[sl], dtype=np.float32),
            "diar": np.ascontiguousarray(diar_probs[sl], dtype=np.float32),
            "lab": np.ascontiguousarray(labels[sl], dtype=np.float32),
        })
    return maps


# ---------------- kernel entry (self-contained) ----------------

N_CORES = 8
_CACHE = {}


def _get_nc():
    if "nc" not in _CACHE:
        _CACHE["nc"] = build_nc_v4(T=128000, diar_front=True)[0]
    return _CACHE["nc"]


def kernel(separated, diar_probs, exist_probs, sources, labels, num_speakers):
    """EEND-SS loss on 8 NeuronCores: batch sharded 4 samples/core; device
    computes the big time-axis Grams; host does the tiny PIT/existence math."""
    from concourse.bass_utils import run_bass_kernel_spmd

    separated = np.asarray(separated)
    diar_probs = np.asarray(diar_probs)
    exist_probs = np.asarray(exist_probs)
    sources = np.asarray(sources)
    labels = np.asarray(labels)
    num_speakers = np.asarray(num_speakers)

    nc = _get_nc()
    in_maps = shard_inputs(separated, diar_probs, sources, labels, N_CORES)
    res = run_bass_kernel_spmd(nc, in_maps, list(range(N_CORES)))

    # gram output [125, 152]: cols 0:125 = chunk-blocked Gram, cols 125:149
    # rows 0:24 carry the diar Gram
    gammas = [host_gamma_fp32(res.results[c]["gram"][:, :PD]) for c in range(N_CORES)]
    dgrams = [res.results[c]["gram"][:NDATA, PD:PD + NDATA] for c in range(N_CORES)]
    return host_finalize(gammas, dgrams, exist_probs, num_speakers, T=128000,
                         tmajor=True)

